# revision 1
# baseline (speedup 1.0000x reference)
"""Fused ViT transformer block on Trainium2, data-parallel over batch across 8 cores.

Per-core shard [4, 577, 1024] (D=1024, 16 heads, MLP 4096). Design notes:

  - All matmul operands are bf16 (1 PE cycle/row regardless of N/M, vs
    fp32r's 4x penalty at N<256 and fp32's 2x transposes). PSUM accumulation,
    the residual stream, and LN statistics stay f32; rel err ~1.5e-3.
  - Batches run in 2 groups of 2. Within a group every phase is weight-major,
    so each weight tile is DMA'd once per group as a single natural-slice
    descriptor from a host-side pre-permuted layout: ~98 DMAs total vs 895 in
    the fp32r baseline (each dma_start costs ~1.3us of shared HWDGE/DGE pipe).
  - Attention computes scores directly transposed, S^T = [k, q], with the key
    tile as the stationary operand, eliminating the baseline's 205k rows/batch
    of probability transposes. exp(S^T) goes to SBUF bf16 via the Act engine.
    AV uses V in [k, d] layout with an appended ones column so one PSUM
    accumulation chain yields both the head output [64, q] and the softmax
    denominators (row 64). A reciprocal row (DVE -> bf16) is broadcast to 64
    partitions by a K=1 PE matmul, Act-evicted to SBUF, and multiplied into
    the output on eviction (DVE quadrant-shifted write for the odd head).
  - Phase B is software-pipelined: AV/normalize of iteration n is emitted
    after QKV/scores of iteration n+1, so the PE does not wait on the Act
    exp stream. Groups' phases are interleaved (grp1's LN1 fills the PE gap
    at grp0's proj->LN2 boundary). wproj is prefetched during attention.
  - MLP hidden blocks run in pairs (8 k-tiles -> one PSUM accumulation per
    1024-col tile, single DVE eviction-add). fc1 bias+gelu fuse into the Act
    eviction. LN affines are folded into adjacent weights host-side; proj_b
    is pre-added to the residual x tiles on the idle GPSIMD engine and fc2_b
    folded in the same way.

Cost-model (TimelineSim): 1,129,313 ns/core (PE busy 981us = 87%);
measured on hw via paired differential bench: ~1.09 ms median
(fp32r baseline: 1,601,115 ns predicted / 2,595,817 ns measured).
"""

import numpy as np
from contextlib import ExitStack

import concourse.bass as bass
import concourse.mybir as mybir
import concourse.tile as tile
from concourse import bacc
from concourse.masks import make_identity

P = 128
F32 = mybir.dt.float32
BF = mybir.dt.bfloat16
AF = mybir.ActivationFunctionType
ALU = mybir.AluOpType


def build_nc(B=4, T=577, D=1024, H=16, HD=64, HID=4096, eps=1e-6):
    assert D % P == 0 and H % 2 == 0 and HD == 64 and HID % 512 == 0
    KD = D // P               # 8 k-tiles over model dim
    NPAIR = H // 2            # 8 head pairs
    NHB = HID // 512          # 8 hidden blocks
    KH = 512 // P             # 4 k-tiles per hidden block
    NHT = HID // P            # 32 fc1 output tiles
    GRP = 2                   # batches per weight-pass group
    NG = B // GRP
    scale = 1.0 / float(np.sqrt(HD))

    t_tiles = [(i, min(P, T - i)) for i in range(0, T, P)]      # exact tiles
    NTT = len(t_tiles)
    n_blocks = [(i, min(512, T - i)) for i in range(0, T, 512)]  # psum-bank cols

    nc = bacc.Bacc(None, target_bir_lowering=False, debug=False)

    x_d = nc.dram_tensor("x", [B * T, D], F32, kind="ExternalInput")
    wqkv_d = nc.dram_tensor("wqkv", [P, NPAIR, KD, 3 * P], BF, kind="ExternalInput")
    cq_d = nc.dram_tensor("cq", [P, NPAIR * 3], F32, kind="ExternalInput")
    wp_d = nc.dram_tensor("wproj", [P, KD, D], BF, kind="ExternalInput")
    cpb_d = nc.dram_tensor("cpb", [P, D], F32, kind="ExternalInput")
    wf1_d = nc.dram_tensor("wfc1", [P, NHB, KD, 512], BF, kind="ExternalInput")
    cf1_d = nc.dram_tensor("cf1", [P, NHT], F32, kind="ExternalInput")
    wf2_d = nc.dram_tensor("wfc2", [P, NHB, KH, D], BF, kind="ExternalInput")
    cf2_d = nc.dram_tensor("cf2", [P, D], F32, kind="ExternalInput")
    out_d = nc.dram_tensor("out", [B * T, D], F32, kind="ExternalOutput")

    with tile.TileContext(nc) as tc, ExitStack() as ctx:
        const = ctx.enter_context(tc.tile_pool(name="const", bufs=1))
        statp = ctx.enter_context(tc.tile_pool(name="stat", bufs=8))
        lnT_p = ctx.enter_context(tc.tile_pool(name="lnT", bufs=4))
        aT_p = ctx.enter_context(tc.tile_pool(name="aT", bufs=2))
        x2_p = ctx.enter_context(tc.tile_pool(name="x2", bufs=2))
        w_p = ctx.enter_context(tc.tile_pool(name="wpool", bufs=2))
        xin_p = ctx.enter_context(tc.tile_pool(name="xin", bufs=2))
        scr_p = ctx.enter_context(tc.tile_pool(name="scr", bufs=3))
        qkvt_p = ctx.enter_context(tc.tile_pool(name="qkvt", bufs=3))
        vkd_p = ctx.enter_context(tc.tile_pool(name="vkd", bufs=3))
        expst_p = ctx.enter_context(tc.tile_pool(name="expst", bufs=4))
        rinv_p = ctx.enter_context(tc.tile_pool(name="rinv", bufs=1))
        hT_p = ctx.enter_context(tc.tile_pool(name="hTp", bufs=2))
        pmm = ctx.enter_context(tc.tile_pool(name="pmm", bufs=2, space="PSUM"))
        pav = ctx.enter_context(tc.tile_pool(name="pav", bufs=2, space="PSUM"))

        ident = const.tile([P, P], F32)
        make_identity(nc, ident)
        identA = const.tile([P, P], BF)
        nc.vector.tensor_copy(out=identA[:, :], in_=ident[:, :])
        eps_t = const.tile([P, 1], F32)
        nc.vector.memset(eps_t, eps)
        ones_bf = const.tile([65, 64], BF)
        nc.vector.memset(ones_bf[64:65, :], 1.0)
        cq_sb = const.tile([P, NPAIR * 3], F32)
        nc.sync.dma_start(out=cq_sb[:, :], in_=cq_d[:, :])
        cf1_sb = const.tile([P, NHT], F32)
        nc.sync.dma_start(out=cf1_sb[:, :], in_=cf1_d[:, :])
        cpb_sb = const.tile([P, D], F32)
        nc.sync.dma_start(out=cpb_sb[:, :], in_=cpb_d[:, :])
        cf2_sb = const.tile([P, D], F32)
        nc.sync.dma_start(out=cf2_sb[:, :], in_=cf2_d[:, :])

        def ln_stats(src, tsz):
            """src: SBUF f32 AP [tsz, D]. Returns (mean, istd) stat tiles."""
            stats = statp.tile([P, 2, 6], F32, tag="bnst")
            xv = src.rearrange("p (s f) -> p s f", s=2)
            for s in range(2):
                nc.vector.bn_stats(out=stats[0:tsz, s, :], in_=xv[:, s, :])
            mv = statp.tile([P, 2], F32, tag="mv")
            nc.vector.bn_aggr(out=mv[0:tsz, :], in_=stats[0:tsz])
            istd = statp.tile([P, 1], F32, tag="istd")
            nc.scalar.activation(out=istd[0:tsz], in_=mv[0:tsz, 1:2],
                                 func=AF.Sqrt, bias=eps_t[0:tsz])
            nc.vector.reciprocal(out=istd[0:tsz], in_=istd[0:tsz])
            return mv, istd

        def ln_norm_tr(src, tsz, negmi, istd, dstT, t0):
            """Normalize src with (mv, istd) -> bf16, PE-transpose into
            dstT[:, :, t0:t0+tsz]."""
            scr = scr_p.tile([P, D], BF)
            nc.vector.tensor_scalar(
                out=scr[0:tsz, :], in0=src,
                scalar1=negmi[0:tsz, 0:1], scalar2=istd[0:tsz],
                op0=ALU.subtract, op1=ALU.mult,
            )
            for kt0 in range(0, KD, 4):
                trp = pav.tile([P, 4, P], BF, tag="av")
                for j4 in range(4):
                    kt = kt0 + j4
                    nc.tensor.matmul(
                        trp[:, j4, 0:tsz], scr[0:tsz, kt * P:(kt + 1) * P],
                        identA[0:tsz, 0:tsz], is_transpose=True,
                        start=(j4 == 0), stop=(j4 == 3),
                    )
                nc.vector.tensor_copy(
                    out=dstT[:, kt0:kt0 + 4, t0:t0 + tsz],
                    in_=trp[:, :, 0:tsz])

        def ln_transpose(src, tsz, dstT, t0):
            negmi, istd = ln_stats(src, tsz)
            ln_norm_tr(src, tsz, negmi, istd, dstT, t0)

        def phase_a(grp):
            """LN1 for both batches of the group -> per-gg xn1T tiles."""
            xn1Ts = []
            for gg in range(GRP):
                g = grp * GRP + gg
                xn1T = lnT_p.tile([P, KD, T], BF, tag="lnT", name=f"xn1T{gg}")
                xn1Ts.append(xn1T)
                for (t0, tsz) in t_tiles:
                    xt = xin_p.tile([P, D], F32)
                    nc.sync.dma_start(out=xt[0:tsz, :],
                                      in_=x_d[g * T + t0:g * T + t0 + tsz, :])
                    ln_transpose(xt[0:tsz, :], tsz, xn1T, t0)
            return xn1Ts

        def attn_stage1(p, gg, wq, xn1T):
            """QKV matmuls + evicts, scores+exp for both heads, V->[k,d]."""
            qT = qkvt_p.tile([P, T], BF, tag="tq")
            kTt = qkvt_p.tile([P, T], BF, tag="tk")
            vT = qkvt_p.tile([P, T], BF, tag="tv")
            for i, dst in enumerate((qT, kTt, vT)):
                ps = pmm.tile([P, T], F32, tag="mm")
                for kt in range(KD):
                    lhsT = wq[:, kt, i * P:(i + 1) * P]
                    for (n0, nsz) in n_blocks:
                        nc.tensor.matmul(
                            ps[:, n0:n0 + nsz], lhsT,
                            xn1T[:, kt, n0:n0 + nsz],
                            start=(kt == 0), stop=(kt == KD - 1),
                        )
                nc.vector.tensor_scalar(
                    out=dst[:, :], in0=ps[:, 0:T],
                    scalar1=cq_sb[:, p * 3 + i:p * 3 + i + 1],
                    scalar2=None, op0=ALU.add)
            expsts = []
            for hi in range(2):
                hb0 = 64 * hi
                expst = expst_p.tile([P, NTT, T], BF, tag="expst")
                for kti, (k0, ksz) in enumerate(t_tiles):
                    st = pmm.tile([P, T], F32, tag="mm")
                    lhsT = kTt[hb0:hb0 + 64, k0:k0 + ksz]
                    for (n0, nsz) in n_blocks:
                        nc.tensor.matmul(st[0:ksz, n0:n0 + nsz], lhsT,
                                         qT[hb0:hb0 + 64, n0:n0 + nsz])
                    nc.scalar.activation(out=expst[0:ksz, kti, :],
                                         in_=st[0:ksz, 0:T],
                                         func=AF.Exp, scale=scale)
                expsts.append(expst)
            vkd2 = vkd_p.tile([P, NTT, 130], BF)
            nc.vector.memset(vkd2[:, :, 64:65], 1.0)
            nc.vector.memset(vkd2[:, :, 129:130], 1.0)
            for kti, (k0, ksz) in enumerate(t_tiles):
                trp = pav.tile([P, P], BF, tag="av")
                nc.tensor.matmul(trp[0:ksz, :], vT[:, k0:k0 + ksz],
                                 identA[:, :], is_transpose=True)
                dstv = vkd2[0:ksz, kti, 0:130].rearrange(
                    "p (s f) -> p s f", s=2)[:, :, 0:64]
                nc.vector.tensor_copy(
                    out=dstv,
                    in_=trp[0:ksz, :].rearrange("p (s f) -> p s f", s=2))
            return (p, gg, expsts, vkd2)

        def attn_stage2(state, aTs):
            """AV with fused denominator row; normalize on eviction."""
            p, gg, expsts, vkd2 = state
            muls = []
            for hi in range(2):
                av = pav.tile([65, T], F32, tag="av")
                for kti, (k0, ksz) in enumerate(t_tiles):
                    lhsT = vkd2[0:ksz, kti, hi * 65:(hi + 1) * 65]
                    for (n0, nsz) in n_blocks:
                        nc.tensor.matmul(
                            av[:, n0:n0 + nsz], lhsT,
                            expsts[hi][0:ksz, kti, n0:n0 + nsz],
                            start=(kti == 0), stop=(kti == NTT - 1),
                        )
                rin = rinv_p.tile([65, T], BF, tag="rin")
                with nc.allow_low_precision(reason="softmax denom bf16"):
                    nc.vector.reciprocal(out=rin[64:65, :],
                                         in_=av[64:65, 0:T])
                bc = pmm.tile([64, T], F32, tag="mm")
                for (n0, nsz) in n_blocks:
                    nc.tensor.matmul(bc[:, n0:n0 + nsz],
                                     ones_bf[64:65, :],
                                     rin[64:65, n0:n0 + nsz])
                bcs = rinv_p.tile([64, T], BF, tag="bcs")
                nc.scalar.copy(out=bcs[:, :], in_=bc[:, 0:T])
                muls.append((av, bcs))
            for hi, (av, bcs) in enumerate(muls):
                hb0 = 64 * hi
                nc.vector.tensor_mul(out=aTs[gg][hb0:hb0 + 64, p, :],
                                     in0=av[0:64, 0:T], in1=bcs[:, 0:T])

        def phase_b(grp, xn1Ts):
            """Attention, software-pipelined: stage2 of iteration n is
            emitted after stage1 of iteration n+1 so the PE never waits
            on the Act exp stream."""
            aTs = [aT_p.tile([P, KD, T], BF, tag="aT", name=f"aT{gg}")
                   for gg in range(GRP)]
            wp = None
            pending = None
            for p in range(NPAIR):
                if p == NPAIR - 1:
                    # prefetch wproj so phase C starts without a DMA stall
                    wp = w_p.tile([P, KD, D], BF, tag="W")
                    nc.sync.dma_start(out=wp[:, :, :], in_=wp_d[:, :, :])
                wq = w_p.tile([P, KD, 3 * P], BF, tag="W")
                nc.sync.dma_start(out=wq[:, :, :], in_=wqkv_d[:, p, :, :])
                for gg in range(GRP):
                    st1 = attn_stage1(p, gg, wq, xn1Ts[gg])
                    if pending is not None:
                        attn_stage2(pending, aTs)
                    pending = st1
            attn_stage2(pending, aTs)
            return aTs, wp

        def phase_c(grp, aTs, wp):
            xn2Ts = []
            x2s_t = []
            for gg in range(GRP):
                g = grp * GRP + gg
                xn2T = lnT_p.tile([P, KD, T], BF, tag="lnT", name=f"xn2T{gg}")
                xn2Ts.append(xn2T)
                x2 = x2_p.tile([P, NTT, D], F32, tag="x2", name=f"x2_{gg}")
                x2s_t.append(x2)
                stats2 = []
                for j, (t0, tsz) in enumerate(t_tiles):
                    xt = xin_p.tile([P, D], F32)
                    nc.sync.dma_start(out=xt[0:tsz, :],
                                      in_=x_d[g * T + t0:g * T + t0 + tsz, :])
                    nc.gpsimd.tensor_add(out=xt[0:tsz, :], in0=xt[0:tsz, :],
                                         in1=cpb_sb[0:tsz, :])
                    ps = pmm.tile([P, D], F32, tag="mm")
                    for kt in range(KD):
                        lhsT = aTs[gg][:, kt, t0:t0 + tsz]
                        for o in range(2):
                            nc.tensor.matmul(
                                ps[0:tsz, o * 512:(o + 1) * 512], lhsT,
                                wp[:, kt, o * 512:(o + 1) * 512],
                                start=(kt == 0), stop=(kt == KD - 1),
                            )
                    x2s = x2[0:tsz, j, :]
                    nc.vector.tensor_add(out=x2s, in0=ps[0:tsz, :], in1=xt[0:tsz, :])
                    stats2.append(ln_stats(x2s, tsz))
                for j, (t0, tsz) in enumerate(t_tiles):
                    x2s = x2[0:tsz, j, :]
                    mv, istd = stats2[j]
                    ln_norm_tr(x2s, tsz, mv, istd, xn2T, t0)
                    nc.gpsimd.tensor_add(out=x2s, in0=x2s, in1=cf2_sb[0:tsz, :])
            return xn2Ts, x2s_t

        def phase_d(grp, xn2Ts, x2s_t):
            for hbp in range(NHB // 2):
                f1 = w_p.tile([P, 2, KD, 512], BF, tag="W")
                nc.sync.dma_start(out=f1[:, :, :, :],
                                  in_=wf1_d[:, 2 * hbp:2 * hbp + 2, :, :])
                f2 = w_p.tile([P, 2, KH, D], BF, tag="W")
                nc.sync.dma_start(out=f2[:, :, :, :],
                                  in_=wf2_d[:, 2 * hbp:2 * hbp + 2, :, :])
                hTs = []
                for gg in range(GRP):
                    hT = hT_p.tile([P, 2 * KH, T], BF)
                    for hb2 in range(2):
                        for ht in range(KH):
                            ps = pmm.tile([P, T], F32, tag="mm")
                            for kt in range(KD):
                                lhsT = f1[:, hb2, kt, ht * P:(ht + 1) * P]
                                for (n0, nsz) in n_blocks:
                                    nc.tensor.matmul(
                                        ps[:, n0:n0 + nsz], lhsT,
                                        xn2Ts[gg][:, kt, n0:n0 + nsz],
                                        start=(kt == 0), stop=(kt == KD - 1),
                                    )
                            hidx = (2 * hbp + hb2) * KH + ht
                            nc.scalar.activation(
                                out=hT[:, hb2 * KH + ht, :], in_=ps[:, 0:T],
                                func=AF.Gelu, bias=cf1_sb[:, hidx:hidx + 1])
                    hTs.append(hT)
                for gg in range(GRP):
                    hT = hTs[gg]
                    for j, (t0, tsz) in enumerate(t_tiles):
                        ps = pmm.tile([P, D], F32, tag="mm")
                        for k8 in range(2 * KH):
                            lhsT = hT[:, k8, t0:t0 + tsz]
                            for o in range(2):
                                nc.tensor.matmul(
                                    ps[0:tsz, o * 512:(o + 1) * 512], lhsT,
                                    f2[:, k8 // KH, k8 % KH, o * 512:(o + 1) * 512],
                                    start=(k8 == 0), stop=(k8 == 2 * KH - 1),
                                )
                        x2s = x2s_t[gg][0:tsz, j, :]
                        nc.vector.tensor_add(out=x2s, in0=x2s, in1=ps[0:tsz, :])
            for gg in range(GRP):
                g = grp * GRP + gg
                for j, (t0, tsz) in enumerate(t_tiles):
                    nc.sync.dma_start(out=out_d[g * T + t0:g * T + t0 + tsz, :],
                                      in_=x2s_t[gg][0:tsz, j, :])

        # Interleaved emission: grp1's LN1 fills the PE gap at grp0's
        # C->D boundary (proj-evict -> stats -> norm chain on DVE).
        a0 = phase_a(0)
        aT0, wp0 = phase_b(0, a0)
        c0 = phase_c(0, aT0, wp0)
        a1 = phase_a(1)
        phase_d(0, *c0)
        aT1, wp1 = phase_b(1, a1)
        c1 = phase_c(1, aT1, wp1)
        phase_d(1, *c1)

    nc.compile()
    return nc


def prepare_inputs(inputs, B, T, D, H, HID, n_cores):
    """Host-side folding + weight permutation into DMA-friendly layouts."""
    f8 = np.float64
    bf16 = mybir.dt.np(BF)
    x = np.asarray(inputs["x"], np.float32)
    g1 = np.asarray(inputs["ln1_g"], f8)
    b1 = np.asarray(inputs["ln1_b"], f8)
    qkv_w = np.asarray(inputs["qkv_w"], f8)
    qkv_b = np.asarray(inputs["qkv_b"], f8)
    proj_w = np.asarray(inputs["proj_w"], np.float32)
    proj_b = np.asarray(inputs["proj_b"], np.float32)
    g2 = np.asarray(inputs["ln2_g"], f8)
    b2 = np.asarray(inputs["ln2_b"], f8)
    fc1_w = np.asarray(inputs["fc1_w"], f8)
    fc1_b = np.asarray(inputs["fc1_b"], f8)
    fc2_w = np.asarray(inputs["fc2_w"], np.float32)
    fc2_b = np.asarray(inputs["fc2_b"], np.float32)

    KD = D // P
    NPAIR = H // 2
    NHB = HID // 512
    KH = 512 // P
    NHT = HID // P

    wq = (g1[:, None] * qkv_w).astype(np.float32)
    cq = (b1 @ qkv_w + qkv_b).astype(np.float32)
    wq_, wk_, wv_ = wq[:, :D], wq[:, D:2 * D], wq[:, 2 * D:]
    Wq = np.stack([
        np.concatenate([wq_[:, p * P:(p + 1) * P], wk_[:, p * P:(p + 1) * P],
                        wv_[:, p * P:(p + 1) * P]], axis=1)
        for p in range(NPAIR)
    ], axis=0)                                            # [NPAIR, D, 384]
    wqkv_h = np.ascontiguousarray(
        Wq.reshape(NPAIR, KD, P, 3 * P).transpose(2, 0, 1, 3)).astype(bf16)
    cq_, ck_, cv_ = cq[:D], cq[D:2 * D], cq[2 * D:]
    cq_t = np.stack([cq_.reshape(NPAIR, P), ck_.reshape(NPAIR, P),
                     cv_.reshape(NPAIR, P)], axis=1)      # [NPAIR, 3, P]
    cq_t = np.ascontiguousarray(cq_t.transpose(2, 0, 1).reshape(P, NPAIR * 3),
                                dtype=np.float32)

    wproj_h = np.ascontiguousarray(
        proj_w.reshape(KD, P, D).transpose(1, 0, 2)).astype(bf16)
    wf1 = (g2[:, None] * fc1_w).astype(np.float32)
    cf1 = (b2 @ fc1_w + fc1_b).astype(np.float32)
    wf1_h = np.ascontiguousarray(
        wf1.reshape(KD, P, NHB, 512).transpose(1, 2, 0, 3)).astype(bf16)
    cf1_t = np.ascontiguousarray(cf1.reshape(NHT, P).T, dtype=np.float32)
    wf2_h = np.ascontiguousarray(
        fc2_w.reshape(NHB, KH, P, D).transpose(2, 0, 1, 3)).astype(bf16)
    cpb_h = np.ascontiguousarray(np.broadcast_to(proj_b, (P, D)), np.float32)
    cf2_h = np.ascontiguousarray(np.broadcast_to(fc2_b, (P, D)), np.float32)

    Bc = B // n_cores
    TOK = Bc * T
    shared = dict(wqkv=wqkv_h, cq=cq_t, wproj=wproj_h, cpb=cpb_h,
                  wfc1=wf1_h, cf1=cf1_t, wfc2=wf2_h, cf2=cf2_h)
    in_maps = []
    for c in range(n_cores):
        m = dict(shared)
        m["x"] = np.ascontiguousarray(x[c * Bc:(c + 1) * Bc].reshape(TOK, D))
        in_maps.append(m)
    return in_maps


_NC_CACHE = {}


def _get_nc(B, T, D, H, HD, HID):
    key = (B, T, D, H, HD, HID)
    if key not in _NC_CACHE:
        _NC_CACHE[key] = build_nc(B=B, T=T, D=D, H=H, HD=HD, HID=HID)
    return _NC_CACHE[key]


def _run(inputs, trace=False):
    from concourse.bass_utils import run_bass_kernel_spmd
    x = np.asarray(inputs["x"])
    B, T, D = x.shape
    H = 16
    HD = D // H
    HID = np.asarray(inputs["fc1_w"]).shape[1]
    n_cores = 8
    Bc = B // n_cores
    nc = _get_nc(Bc, T, D, H, HD, HID)
    in_maps = prepare_inputs(inputs, B, T, D, H, HID, n_cores)
    res = run_bass_kernel_spmd(nc, in_maps, list(range(n_cores)), trace=trace)
    out = np.concatenate(
        [res.results[c]["out"].reshape(Bc, T, D) for c in range(n_cores)], axis=0)
    return out, res


def kernel(**inputs) -> np.ndarray:
    out, _ = _run(inputs, trace=False)
    return out.astype(np.float32)



# revision 25
# speedup vs baseline: 1.0694x; 1.0694x over previous
"""Fused ViT transformer block on Trainium2, data-parallel over batch across 8 cores.

Per-core shard [4, 577, 1024] (D=1024, 16 heads, MLP 4096). Design notes:

  - Weight matmuls (qkv/proj/fc1/fc2) run in fp8 DoubleRow mode at 0.5
    PE cycles/row (2x bf16): each weight is stored as an e4m3 (hi, lo)
    pair with hi+lo == bf16-accurate w*32, and the activation operand is
    broadcast (stride-0 AP) into both pair slots, so the PE computes
    w_hi*x + w_lo*x = w*x with full weight precision. Only the one e4m3
    activation quantization (~3.6% RMS/el) enters the error budget; the
    1/32 scale folds into the (existing) eviction ops.
  - AV also runs DoubleRow by pairing adjacent key tiles (both operands
    genuinely fp8: exp probabilities and V), with the trailing 65-row key
    tile as a plain fp8 matmul. Scores stay bf16 (K=64 cannot pair).
    PSUM accumulation, residual stream, LN stats stay f32.
  - Batches run in 2 groups of 2, weight-major within a group; each
    weight tile is DMA'd once per group as a single natural-slice
    descriptor from a host-side pre-permuted hi/lo-interleaved layout.
  - Attention computes scores directly transposed, S^T = [k, q]; exp(S^T)
    goes to SBUF fp8 via Act. AV uses V in [k, d] fp8 layout with an
    appended ones column so one PSUM chain yields the head output and the
    softmax denominators; a bf16 reciprocal row is broadcast by a K=1
    matmul and multiplied in on eviction (aT written fp8 for proj).
  - Phase B is software-pipelined (AV/normalize of iter n after
    QKV/scores of n+1); groups' phases interleave; wproj prefetched.
  - fc1 bias+gelu fuse into the Act eviction (scale=1/32); proj/fc2
    evictions use scalar_tensor_tensor (psum/32 + residual) on DVE.
"""

import numpy as np
from contextlib import ExitStack

import concourse.bass as bass
import concourse.mybir as mybir
import concourse.tile as tile
from concourse import bacc
from concourse.masks import make_identity

P = 128
F32 = mybir.dt.float32
BF = mybir.dt.bfloat16
F8 = mybir.dt.float8e4
AF = mybir.ActivationFunctionType
ALU = mybir.AluOpType
DR = mybir.MatmulPerfMode.DoubleRow

WS = 32.0          # weight pre-scale before e4m3 hi/lo split
QKV_DR = True      # xn1 site
PROJ_DR = True     # aT site
FC1_DR = True      # xn2 site
FC2_DR = True      # hT site
AV_F8 = True


def build_nc(B=4, T=577, D=1024, H=16, HD=64, HID=4096, eps=1e-6):
    assert D % P == 0 and H % 2 == 0 and HD == 64 and HID % 512 == 0
    KD = D // P               # 8 k-tiles over model dim
    NPAIR = H // 2            # 8 head pairs
    NHB = HID // 512          # 8 hidden blocks
    KH = 512 // P             # 4 k-tiles per hidden block
    NHT = HID // P            # 32 fc1 output tiles
    GRP = 2                   # batches per weight-pass group
    NG = B // GRP
    scale = 1.0 / float(np.sqrt(HD))

    t_tiles = [(i, min(P, T - i)) for i in range(0, T, P)]      # exact tiles
    NTT = len(t_tiles)
    n_blocks = [(i, min(512, T - i)) for i in range(0, T, 512)]  # psum-bank cols
    nch_T = [(i, min(256, T - i)) for i in range(0, T, 256)]     # DR chunks
    TP = ((T + 15) // 16) * 16   # fp8 row pitch: 2B/16B alignment for DR APs
    nch_D = [(i, 256) for i in range(0, D, 256)]

    XN_DT = F8 if QKV_DR else BF      # xn1T dtype
    XN2_DT = F8 if FC1_DR else BF
    AT_DT = F8 if PROJ_DR else BF
    HT_DT = F8 if FC2_DR else BF
    AV_DT = F8 if AV_F8 else BF

    nc = bacc.Bacc(None, target_bir_lowering=False, debug=False)

    x_d = nc.dram_tensor("x", [B * T, D], F32, kind="ExternalInput")
    wqkv_d = nc.dram_tensor(
        "wqkv", [P, NPAIR, KD, 2, 3 * P] if QKV_DR else [P, NPAIR, KD, 3 * P],
        F8 if QKV_DR else BF, kind="ExternalInput")
    cq_d = nc.dram_tensor("cq", [P, NPAIR * 3], F32, kind="ExternalInput")
    wp_d = nc.dram_tensor(
        "wproj", [P, KD, 2, D] if PROJ_DR else [P, KD, D],
        F8 if PROJ_DR else BF, kind="ExternalInput")
    cpb_d = nc.dram_tensor("cpb", [P, D], F32, kind="ExternalInput")
    cvb_d = nc.dram_tensor("cvb", [P, H // 2, P], F32, kind="ExternalInput")
    wf1_d = nc.dram_tensor(
        "wfc1", [P, NHB, KD, 2, 512] if FC1_DR else [P, NHB, KD, 512],
        F8 if FC1_DR else BF, kind="ExternalInput")
    cf1_d = nc.dram_tensor("cf1", [P, NHT], F32, kind="ExternalInput")
    wf2_d = nc.dram_tensor(
        "wfc2", [P, NHB, KH, 2, D] if FC2_DR else [P, NHB, KH, D],
        F8 if FC2_DR else BF, kind="ExternalInput")
    cf2_d = nc.dram_tensor("cf2", [P, D], F32, kind="ExternalInput")
    out_d = nc.dram_tensor("out", [B * T, D], F32, kind="ExternalOutput")

    def pair(ap, n):
        """Broadcast a [p, n] AP into both DoubleRow pair slots."""
        return ap.unsqueeze(1).broadcast_to([ap.shape[0], 2, n])

    with tile.TileContext(nc) as tc, ExitStack() as ctx:
        const = ctx.enter_context(tc.tile_pool(name="const", bufs=1))
        statp = ctx.enter_context(tc.tile_pool(name="stat", bufs=8))
        lnT_p = ctx.enter_context(tc.tile_pool(name="lnT", bufs=4))
        aT_p = ctx.enter_context(tc.tile_pool(name="aT", bufs=2))
        x2_p = ctx.enter_context(tc.tile_pool(name="x2", bufs=2))
        w_p = ctx.enter_context(tc.tile_pool(name="wpool", bufs=2))
        w2_p = ctx.enter_context(tc.tile_pool(name="wpool2", bufs=2))
        xin_p = ctx.enter_context(tc.tile_pool(name="xin", bufs=2))
        scr_p = ctx.enter_context(tc.tile_pool(name="scr", bufs=3))
        qkvt_p = ctx.enter_context(tc.tile_pool(name="qkvt", bufs=3))
        vkd_p = ctx.enter_context(tc.tile_pool(name="vkd", bufs=3))
        expst_p = ctx.enter_context(tc.tile_pool(name="expst", bufs=4))
        rinv_p = ctx.enter_context(tc.tile_pool(name="rinv", bufs=2))
        hT_p = ctx.enter_context(tc.tile_pool(name="hTp", bufs=2))
        pmm = ctx.enter_context(tc.tile_pool(name="pmm", bufs=2, space="PSUM"))
        pst = ctx.enter_context(tc.tile_pool(name="pst", bufs=2, space="PSUM"))

        ident = const.tile([P, P], F32)
        make_identity(nc, ident)
        identA = const.tile([P, P], BF)
        nc.vector.tensor_copy(out=identA[:, :], in_=ident[:, :])
        eps_t = const.tile([P, 1], F32)
        nc.vector.memset(eps_t, eps)
        ones_bf = const.tile([65, 64], BF)
        nc.vector.memset(ones_bf[64:65, :], 1.0)
        cq_sb = const.tile([P, NPAIR * 3], F32)
        nc.sync.dma_start(out=cq_sb[:, :], in_=cq_d[:, :])
        cf1_sb = const.tile([P, NHT], F32)
        nc.sync.dma_start(out=cf1_sb[:, :], in_=cf1_d[:, :])
        cpb_sb = const.tile([P, D], F32)
        nc.sync.dma_start(out=cpb_sb[:, :], in_=cpb_d[:, :])
        cvb_sb = const.tile([P, NPAIR, P], F32)
        nc.sync.dma_start(out=cvb_sb[:, :, :], in_=cvb_d[:, :, :])
        cf2_sb = const.tile([P, D], F32)
        nc.sync.dma_start(out=cf2_sb[:, :], in_=cf2_d[:, :])

        def ln_stats(src, tsz):
            """src: SBUF f32 AP [tsz, D]. Returns (mean, istd) stat tiles."""
            stats = statp.tile([P, 2, 6], F32, tag="bnst")
            xv = src.rearrange("p (s f) -> p s f", s=2)
            for s in range(2):
                nc.vector.bn_stats(out=stats[0:tsz, s, :], in_=xv[:, s, :])
            mv = statp.tile([P, 2], F32, tag="mv")
            nc.vector.bn_aggr(out=mv[0:tsz, :], in_=stats[0:tsz])
            istd = statp.tile([P, 1], F32, tag="istd")
            nc.scalar.activation(out=istd[0:tsz], in_=mv[0:tsz, 1:2],
                                 func=AF.Sqrt, bias=eps_t[0:tsz])
            nc.vector.reciprocal(out=istd[0:tsz], in_=istd[0:tsz])
            return mv, istd

        def ln_norm_tr(src, tsz, negmi, istd, dstT, t0):
            """Normalize src with (mv, istd) -> bf16, PE-transpose into
            dstT[:, :, t0:t0+tsz] (dstT dtype may be fp8; DVE converts)."""
            scr = scr_p.tile([P, D], BF)
            nc.vector.tensor_scalar(
                out=scr[0:tsz, :], in0=src,
                scalar1=negmi[0:tsz, 0:1], scalar2=istd[0:tsz],
                op0=ALU.subtract, op1=ALU.mult,
            )
            for kt0 in range(0, KD, 4):
                trp = pst.tile([P, 4, P], BF, tag="st")
                for j4 in range(4):
                    kt = kt0 + j4
                    nc.tensor.matmul(
                        trp[:, j4, 0:tsz], scr[0:tsz, kt * P:(kt + 1) * P],
                        identA[0:tsz, 0:tsz], is_transpose=True,
                        start=(j4 == 0), stop=(j4 == 3),
                    )
                nc.vector.tensor_copy(
                    out=dstT[:, kt0:kt0 + 4, t0:t0 + tsz],
                    in_=trp[:, :, 0:tsz])

        def ln_transpose(src, tsz, dstT, t0):
            negmi, istd = ln_stats(src, tsz)
            ln_norm_tr(src, tsz, negmi, istd, dstT, t0)

        def phase_a(grp):
            """LN1 for both batches of the group -> per-gg xn1T tiles."""
            xn1Ts = []
            for gg in range(GRP):
                g = grp * GRP + gg
                xn1T = lnT_p.tile([P, KD, TP], XN_DT, tag="lnT", name=f"xn1T{gg}")
                xn1Ts.append(xn1T)
                for (t0, tsz) in t_tiles:
                    xt = xin_p.tile([P, D], F32)
                    nc.sync.dma_start(out=xt[0:tsz, :],
                                      in_=x_d[g * T + t0:g * T + t0 + tsz, :])
                    ln_transpose(xt[0:tsz, :], tsz, xn1T, t0)
            return xn1Ts

        def phase_b_steps(grp, xn1Ts, out):
            """Attention with a 3-deep software pipeline. Macro-step n emits
            the scores/exp stream of iteration n-1 (Act-paced) and fills the
            in-order PE between score steps with the QKV/V chains of
            iteration n and the AV/normalize pieces of iteration n-2."""
            aTs = [aT_p.tile([P, KD, TP], AT_DT, tag="aT", name=f"aT{gg}")
                   for gg in range(GRP)]
            NIT = NPAIR * GRP
            wqs = {}
            states = {}
            wp_box = []

            def qkv_fillers(n):
                p, gg = n // GRP, n % GRP
                if gg == 0:
                    wq = w_p.tile([P, KD, 2, 3 * P], F8, tag="W")
                    nc.sync.dma_start(out=wq[:, :, :, :],
                                      in_=wqkv_d[:, p, :, :, :])
                    wqs[p] = wq
                    if p == NPAIR - 1:
                        wpt = w_p.tile([P, KD, 2, D], F8, tag="W")
                        nc.sync.dma_start(out=wpt[:, :, :, :],
                                          in_=wp_d[:, :, :, :])
                        wp_box.append(wpt)
                wq = wqs[p]
                xn1T = xn1Ts[gg]
                qT = qkvt_p.tile([P, T], BF, tag="tq")
                kT = qkvt_p.tile([P, T], BF, tag="tk")
                vkd2 = vkd_p.tile([P, NTT, 144], AV_DT)
                env = {}

                def qk_chunk(i, dst, kt_lo, kt_hi):
                    def f():
                        if kt_lo == 0:
                            env[i] = pmm.tile([P, T], F32, tag="mm", name=f"qk{i}")
                        ps = env[i]
                        for kt in range(kt_lo, kt_hi):
                            lhsT = wq[:, kt, :, i * P:(i + 1) * P]
                            for (n0, nsz) in nch_T:
                                # start only on the bank-first chunk: a PSUM
                                # start zeroes the whole 2KB bank, so later
                                # chunks in the same bank must not re-start
                                nc.tensor.matmul(
                                    ps[:, n0:n0 + nsz], lhsT,
                                    pair(xn1T[:, kt, n0:n0 + nsz], nsz),
                                    start=(kt == 0 and n0 % 512 == 0),
                                    stop=(kt == KD - 1),
                                    perf_mode=DR)
                        if kt_hi == KD:
                            nc.vector.tensor_scalar(
                                out=dst[:, :], in0=ps[:, 0:T],
                                scalar1=cq_sb[:, p * 3 + i:p * 3 + i + 1],
                                scalar2=1.0 / WS, op0=ALU.add, op1=ALU.mult)
                    return f

                def v_chunk(kti_lo, kti_hi):
                    def f():
                        if kti_lo == 0:
                            env["v"] = pmm.tile([P, NTT, P], F32, tag="mm", name="psv")
                            nc.gpsimd.memset(vkd2[:, :, 64:65], 1.0)
                            nc.gpsimd.memset(vkd2[:, :, 136:137], 1.0)
                        psv = env["v"]
                        for kti in range(kti_lo, kti_hi):
                            k0, ksz = t_tiles[kti]
                            for kt in range(KD):
                                nc.tensor.matmul(
                                    psv[0:ksz, kti, 0:P],
                                    pair(xn1T[:, kt, k0:k0 + ksz], ksz),
                                    wq[:, kt, :, 2 * P:3 * P],
                                    start=(kt == 0 and kti in (0, 4)),
                                    stop=(kt == KD - 1),
                                    perf_mode=DR)
                        if kti_hi == NTT:
                            dstv = vkd2[0:P, 0:NTT, 0:144].rearrange(
                                "p k (s f) -> p k s f", s=2)[:, :, :, 0:64]
                            nc.vector.scalar_tensor_tensor(
                                out=dstv,
                                in0=psv[0:P, :, :].rearrange(
                                    "p k (s f) -> p k s f", s=2),
                                scalar=1.0 / WS,
                                in1=cvb_sb[0:P, p].rearrange(
                                    "p (s f) -> p s f", s=2).unsqueeze(1)
                                    .broadcast_to([P, NTT, 2, 64]),
                                op0=ALU.mult, op1=ALU.add)
                    return f

                states[n] = [p, gg, qT, kT, vkd2, None]
                return [qk_chunk(0, qT, 0, 3), qk_chunk(0, qT, 3, 6),
                        qk_chunk(0, qT, 6, 8),
                        qk_chunk(1, kT, 0, 3), qk_chunk(1, kT, 3, 6),
                        qk_chunk(1, kT, 6, 8),
                        v_chunk(0, 2), v_chunk(2, 4), v_chunk(4, 5)]

            def emit_scores(n, fillers):
                p, gg, qT, kT, vkd2, _ = states[n]
                expsts = []
                for hi in range(2):
                    hb0 = 64 * hi
                    expst = expst_p.tile([P, NTT, TP], AV_DT, tag="expst")
                    for kti, (k0, ksz) in enumerate(t_tiles):
                        st = pst.tile([P, T], F32, tag="st")
                        lhsT = kT[hb0:hb0 + 64, k0:k0 + ksz]
                        for (n0, nsz) in n_blocks:
                            nc.tensor.matmul(st[0:ksz, n0:n0 + nsz], lhsT,
                                             qT[hb0:hb0 + 64, n0:n0 + nsz])
                        nc.scalar.activation(out=expst[0:ksz, kti, 0:T],
                                             in_=st[0:ksz, 0:T],
                                             func=AF.Exp, scale=scale)
                        if fillers:
                            fillers.pop(0)()
                    expsts.append(expst)
                states[n][5] = expsts

            def stage2_pieces(n):
                p, gg, qT, kT, vkd2, expsts = states[n]
                avs = {}

                def av_mms(hi):
                    def f():
                        av = pmm.tile([65, T], F32, tag="mm")
                        avs[hi] = av
                        for kp in (0, 2):
                            lhsT = vkd2[0:P, kp:kp + 2,
                                        hi * 72:hi * 72 + 65]
                            for (n0, nsz) in nch_T:
                                nc.tensor.matmul(
                                    av[:, n0:n0 + nsz], lhsT,
                                    expsts[hi][0:P, kp:kp + 2, n0:n0 + nsz],
                                    start=(kp == 0 and n0 % 512 == 0),
                                    stop=False, perf_mode=DR)
                        k0, ksz = t_tiles[NTT - 1]
                        lhsT = vkd2[0:ksz, NTT - 1, hi * 72:hi * 72 + 65]
                        for (n0, nsz) in nch_T:
                            nc.tensor.matmul(
                                av[:, n0:n0 + nsz], lhsT,
                                expsts[hi][0:ksz, NTT - 1, n0:n0 + nsz],
                                start=False, stop=(n0 == nch_T[-1][0]))
                    return f

                def post(hi):
                    def f():
                        av = avs[hi]
                        rin = rinv_p.tile([65, T], BF, tag="rin")
                        with nc.allow_low_precision(reason="softmax denom"):
                            nc.vector.reciprocal(out=rin[64:65, :],
                                                 in_=av[64:65, 0:T])
                        bc = pst.tile([64, T], F32, tag="st")
                        for (n0, nsz) in n_blocks:
                            nc.tensor.matmul(bc[:, n0:n0 + nsz],
                                             ones_bf[64:65, :],
                                             rin[64:65, n0:n0 + nsz])
                        bcs = rinv_p.tile([64, T], BF, tag="bcs")
                        nc.vector.tensor_copy(out=bcs[:, :], in_=bc[:, 0:T])
                        nc.vector.tensor_mul(
                            out=aTs[gg][64 * hi:64 * hi + 64, p, 0:T],
                            in0=av[0:64, 0:T], in1=bcs[:, 0:T])
                    return f

                return [av_mms(0), post(0), av_mms(1), post(1)]

            for ms in range(NIT + 2):
                fillers = []
                if ms >= 2:
                    p1, p2, p3, p4 = stage2_pieces(ms - 2)
                    fillers += [p1, p3, p2, p4]
                if ms < NIT:
                    fillers += qkv_fillers(ms)
                if 1 <= ms <= NIT:
                    emit_scores(ms - 1, fillers)
                for f in fillers:
                    f()
                # free per-iteration state two steps back
                states.pop(ms - 2, None)
                yield
            out.append((aTs, wp_box[0]))
            yield

        def phase_b(grp, xn1Ts):
            out = []
            for _ in phase_b_steps(grp, xn1Ts, out):
                pass
            return out[0]

        def phase_c(grp, aTs, wp):
            xn2Ts = []
            x2s_t = []
            for gg in range(GRP):
                g = grp * GRP + gg
                xn2T = lnT_p.tile([P, KD, TP], XN2_DT, tag="lnT", name=f"xn2T{gg}")
                xn2Ts.append(xn2T)
                x2 = x2_p.tile([P, NTT, D], F32, tag="x2", name=f"x2_{gg}")
                x2s_t.append(x2)
                stats2 = []
                for j, (t0, tsz) in enumerate(t_tiles):
                    xt = xin_p.tile([P, D], F32)
                    nc.sync.dma_start(out=xt[0:tsz, :],
                                      in_=x_d[g * T + t0:g * T + t0 + tsz, :])
                    nc.gpsimd.tensor_add(out=xt[0:tsz, :], in0=xt[0:tsz, :],
                                         in1=cpb_sb[0:tsz, :])
                    ps = pmm.tile([P, D], F32, tag="mm")
                    for kt in range(KD):
                        if PROJ_DR:
                            lhsT = pair(aTs[gg][:, kt, t0:t0 + tsz], tsz)
                            for (c0, csz) in nch_D:
                                nc.tensor.matmul(
                                    ps[0:tsz, c0:c0 + csz], lhsT,
                                    wp[:, kt, :, c0:c0 + csz],
                                    start=(kt == 0 and c0 % 512 == 0),
                                    stop=(kt == KD - 1),
                                    perf_mode=DR,
                                )
                        else:
                            lhsT = aTs[gg][:, kt, t0:t0 + tsz]
                            for o in range(2):
                                nc.tensor.matmul(
                                    ps[0:tsz, o * 512:(o + 1) * 512], lhsT,
                                    wp[:, kt, o * 512:(o + 1) * 512],
                                    start=(kt == 0), stop=(kt == KD - 1),
                                )
                    x2s = x2[0:tsz, j, :]
                    if PROJ_DR:
                        nc.vector.scalar_tensor_tensor(
                            out=x2s, in0=ps[0:tsz, :], scalar=1.0 / WS,
                            in1=xt[0:tsz, :], op0=ALU.mult, op1=ALU.add)
                    else:
                        nc.vector.tensor_add(out=x2s, in0=ps[0:tsz, :],
                                             in1=xt[0:tsz, :])
                    stats2.append(ln_stats(x2s, tsz))
                for j, (t0, tsz) in enumerate(t_tiles):
                    x2s = x2[0:tsz, j, :]
                    mv, istd = stats2[j]
                    ln_norm_tr(x2s, tsz, mv, istd, xn2T, t0)
                    nc.gpsimd.tensor_add(out=x2s, in0=x2s, in1=cf2_sb[0:tsz, :])
            return xn2Ts, x2s_t

        def phase_d_steps(grp, xn2Ts, x2s_t):
            """MLP; generator yielding after each fc1/fc2 sub-chunk so the
            driver can interleave the (Act-bound) attention of the next
            group with this (PE-bound) phase."""
            for hbp in range(NHB // 2):
                f1 = w2_p.tile([P, 2, KD, 2, 512] if FC1_DR else [P, 2, KD, 512],
                              F8 if FC1_DR else BF, tag="W")
                if FC1_DR:
                    nc.sync.dma_start(out=f1[:, :, :, :, :],
                                      in_=wf1_d[:, 2 * hbp:2 * hbp + 2, :, :, :])
                else:
                    nc.sync.dma_start(out=f1[:, :, :, :],
                                      in_=wf1_d[:, 2 * hbp:2 * hbp + 2, :, :])
                f2 = w2_p.tile([P, 2, KH, 2, D] if FC2_DR else [P, 2, KH, D],
                              F8 if FC2_DR else BF, tag="W")
                if FC2_DR:
                    nc.sync.dma_start(out=f2[:, :, :, :, :],
                                      in_=wf2_d[:, 2 * hbp:2 * hbp + 2, :, :, :])
                else:
                    nc.sync.dma_start(out=f2[:, :, :, :],
                                      in_=wf2_d[:, 2 * hbp:2 * hbp + 2, :, :])
                hTs = []
                for gg in range(GRP):
                    hT = hT_p.tile([P, 2 * KH, TP], HT_DT)
                    for hb2 in range(2):
                        for ht in range(KH):
                            ps = pmm.tile([P, T], F32, tag="mm")
                            for kt in range(KD):
                                if FC1_DR:
                                    lhsT = f1[:, hb2, kt, :, ht * P:(ht + 1) * P]
                                    for (n0, nsz) in nch_T:
                                        nc.tensor.matmul(
                                            ps[:, n0:n0 + nsz], lhsT,
                                            pair(xn2Ts[gg][:, kt, n0:n0 + nsz], nsz),
                                            start=(kt == 0 and n0 % 512 == 0),
                                            stop=(kt == KD - 1),
                                            perf_mode=DR,
                                        )
                                else:
                                    lhsT = f1[:, hb2, kt, ht * P:(ht + 1) * P]
                                    for (n0, nsz) in n_blocks:
                                        nc.tensor.matmul(
                                            ps[:, n0:n0 + nsz], lhsT,
                                            xn2Ts[gg][:, kt, n0:n0 + nsz],
                                            start=(kt == 0), stop=(kt == KD - 1),
                                        )
                            hidx = (2 * hbp + hb2) * KH + ht
                            nc.scalar.activation(
                                out=hT[:, hb2 * KH + ht, 0:T], in_=ps[:, 0:T],
                                func=AF.Gelu, bias=cf1_sb[:, hidx:hidx + 1],
                                scale=(1.0 / WS) if FC1_DR else 1.0)
                    hTs.append(hT)
                    yield
                for gg in range(GRP):
                    hT = hTs[gg]
                    for j, (t0, tsz) in enumerate(t_tiles):
                        ps = pmm.tile([P, D], F32, tag="mm")
                        for k8 in range(2 * KH):
                            if FC2_DR:
                                lhsT = pair(hT[:, k8, t0:t0 + tsz], tsz)
                                for (c0, csz) in nch_D:
                                    nc.tensor.matmul(
                                        ps[0:tsz, c0:c0 + csz], lhsT,
                                        f2[:, k8 // KH, k8 % KH, :, c0:c0 + csz],
                                        start=(k8 == 0 and c0 % 512 == 0),
                                        stop=(k8 == 2 * KH - 1),
                                        perf_mode=DR,
                                    )
                            else:
                                lhsT = hT[:, k8, t0:t0 + tsz]
                                for o in range(2):
                                    nc.tensor.matmul(
                                        ps[0:tsz, o * 512:(o + 1) * 512], lhsT,
                                        f2[:, k8 // KH, k8 % KH, o * 512:(o + 1) * 512],
                                        start=(k8 == 0), stop=(k8 == 2 * KH - 1),
                                    )
                        x2s = x2s_t[gg][0:tsz, j, :]
                        if FC2_DR:
                            nc.vector.scalar_tensor_tensor(
                                out=x2s, in0=ps[0:tsz, :], scalar=1.0 / WS,
                                in1=x2s, op0=ALU.mult, op1=ALU.add)
                        else:
                            nc.vector.tensor_add(out=x2s, in0=x2s,
                                                 in1=ps[0:tsz, :])
                    yield
            for gg in range(GRP):
                g = grp * GRP + gg
                for j, (t0, tsz) in enumerate(t_tiles):
                    nc.sync.dma_start(out=out_d[g * T + t0:g * T + t0 + tsz, :],
                                      in_=x2s_t[gg][0:tsz, j, :])

        def phase_d(grp, xn2Ts, x2s_t):
            for _ in phase_d_steps(grp, xn2Ts, x2s_t):
                pass

        # Interleaved emission: grp1's LN1 fills the PE gap at grp0's
        # C->D boundary; d0 (PE-bound MLP) is emission-interleaved with b1
        # (Act-bound attention) so the exp stream hides under fc1/fc2.
        marks = []

        def mark(label):
            marks.append((label, nc.get_next_instruction_name()))

        mark("a0"); a0 = phase_a(0)
        mark("b0"); aT0, wp0 = phase_b(0, a0)
        mark("c0"); c0 = phase_c(0, aT0, wp0)
        mark("a1"); a1 = phase_a(1)
        mark("d0"); phase_d(0, *c0)
        mark("b1"); aT1, wp1 = phase_b(1, a1)
        mark("c1"); c1 = phase_c(1, aT1, wp1)
        mark("d1"); phase_d(1, *c1)
        mark("end")
        nc._phase_marks = marks

    nc.compile()
    return nc


def _hilo(w, scale, bf16np):
    """e4m3 (hi, lo) split of w*scale along a new last-position axis pair."""
    import ml_dtypes
    E4 = ml_dtypes.float8_e4m3
    ws = np.clip(np.asarray(w, np.float64) * scale, -240, 240)
    hi = ws.astype(E4)
    lo = (ws - hi.astype(np.float64)).astype(E4)
    return hi, lo


def prepare_inputs(inputs, B, T, D, H, HID, n_cores):
    """Host-side folding + weight permutation into DMA-friendly layouts."""
    import ml_dtypes
    E4 = ml_dtypes.float8_e4m3
    f8 = np.float64
    bf16 = mybir.dt.np(BF)
    x = np.asarray(inputs["x"], np.float32)
    g1 = np.asarray(inputs["ln1_g"], f8)
    b1 = np.asarray(inputs["ln1_b"], f8)
    qkv_w = np.asarray(inputs["qkv_w"], f8)
    qkv_b = np.asarray(inputs["qkv_b"], f8)
    proj_w = np.asarray(inputs["proj_w"], f8)
    proj_b = np.asarray(inputs["proj_b"], np.float32)
    g2 = np.asarray(inputs["ln2_g"], f8)
    b2 = np.asarray(inputs["ln2_b"], f8)
    fc1_w = np.asarray(inputs["fc1_w"], f8)
    fc1_b = np.asarray(inputs["fc1_b"], f8)
    fc2_w = np.asarray(inputs["fc2_w"], f8)
    fc2_b = np.asarray(inputs["fc2_b"], np.float32)

    KD = D // P
    NPAIR = H // 2
    NHB = HID // 512
    KH = 512 // P
    NHT = HID // P

    def split_pairs(w, dr):
        """w: [Din, Dout] logical weights. Returns array with Din split into
        [KDin, P] k-tiles; if dr, adds (hi,lo) e4m3 pair axis after k-tile:
        [P(part), KDin, 2, Dout]; else bf16 [P(part), KDin, Dout]."""
        Din, Dout = w.shape
        kd = Din // P
        wt = w.reshape(kd, P, Dout)
        if dr:
            hi, lo = _hilo(wt, WS, bf16)
            pairs = np.stack([hi, lo], axis=2)          # [kd, P, 2, Dout]
            return np.ascontiguousarray(
                pairs.transpose(1, 0, 2, 3))            # [P, kd, 2, Dout]
        return np.ascontiguousarray(
            wt.transpose(1, 0, 2)).astype(bf16)         # [P, kd, Dout]

    wq = g1[:, None] * qkv_w
    cq = (b1 @ qkv_w + qkv_b).astype(np.float64)
    cv_unscaled = cq[2 * D:].copy()
    wq_, wk_, wv_ = wq[:, :D], wq[:, D:2 * D], wq[:, 2 * D:]
    Wq = np.stack([
        np.concatenate([wq_[:, p * P:(p + 1) * P], wk_[:, p * P:(p + 1) * P],
                        wv_[:, p * P:(p + 1) * P]], axis=1)
        for p in range(NPAIR)
    ], axis=0)                                            # [NPAIR, D, 384]
    if QKV_DR:
        hi, lo = _hilo(Wq.reshape(NPAIR, KD, P, 3 * P), WS, bf16)
        pairs = np.stack([hi, lo], axis=3)               # [NPAIR, KD, P, 2, 384]
        wqkv_h = np.ascontiguousarray(pairs.transpose(2, 0, 1, 3, 4))
        cq = cq * WS
    else:
        wqkv_h = np.ascontiguousarray(
            Wq.reshape(NPAIR, KD, P, 3 * P).transpose(2, 0, 1, 3)).astype(bf16)
    cq_, ck_, cv_ = cq[:D], cq[D:2 * D], cq[2 * D:]
    cq_t = np.stack([cq_.reshape(NPAIR, P), ck_.reshape(NPAIR, P),
                     cv_.reshape(NPAIR, P)], axis=1)      # [NPAIR, 3, P]
    cq_t = np.ascontiguousarray(cq_t.transpose(2, 0, 1).reshape(P, NPAIR * 3),
                                dtype=np.float32)

    cv_pairs = cv_unscaled.reshape(NPAIR, P).astype(np.float32)
    cvb_h = np.ascontiguousarray(
        np.broadcast_to(cv_pairs[None, :, :], (P, NPAIR, P)), np.float32)

    wproj_h = split_pairs(proj_w, PROJ_DR)

    wf1 = g2[:, None] * fc1_w
    cf1 = (b2 @ fc1_w + fc1_b).astype(np.float32)
    if FC1_DR:
        w5 = wf1.reshape(KD, P, NHB, 512)
        hi, lo = _hilo(w5, WS, bf16)
        pairs = np.stack([hi, lo], axis=3)               # [KD, P, NHB, 2, 512]
        wf1_h = np.ascontiguousarray(pairs.transpose(1, 2, 0, 3, 4))
    else:
        wf1_h = np.ascontiguousarray(
            wf1.reshape(KD, P, NHB, 512).transpose(1, 2, 0, 3)).astype(bf16)
    cf1_t = np.ascontiguousarray(cf1.reshape(NHT, P).T, dtype=np.float32)

    if FC2_DR:
        w5 = fc2_w.reshape(NHB, KH, P, D)
        hi, lo = _hilo(w5, WS, bf16)
        pairs = np.stack([hi, lo], axis=3)               # [NHB, KH, P, 2, D]
        wf2_h = np.ascontiguousarray(pairs.transpose(2, 0, 1, 3, 4))
    else:
        wf2_h = np.ascontiguousarray(
            fc2_w.reshape(NHB, KH, P, D).transpose(2, 0, 1, 3)).astype(bf16)
    cpb_h = np.ascontiguousarray(np.broadcast_to(proj_b, (P, D)), np.float32)
    cf2_h = np.ascontiguousarray(np.broadcast_to(fc2_b, (P, D)), np.float32)

    Bc = B // n_cores
    TOK = Bc * T
    shared = dict(wqkv=wqkv_h, cq=cq_t, wproj=wproj_h, cpb=cpb_h, cvb=cvb_h,
                  wfc1=wf1_h, cf1=cf1_t, wfc2=wf2_h, cf2=cf2_h)
    in_maps = []
    for c in range(n_cores):
        m = dict(shared)
        m["x"] = np.ascontiguousarray(x[c * Bc:(c + 1) * Bc].reshape(TOK, D))
        in_maps.append(m)
    return in_maps


_NC_CACHE = {}


def _get_nc(B, T, D, H, HD, HID):
    key = (B, T, D, H, HD, HID)
    if key not in _NC_CACHE:
        _NC_CACHE[key] = build_nc(B=B, T=T, D=D, H=H, HD=HD, HID=HID)
    return _NC_CACHE[key]


def _run(inputs, trace=False):
    from concourse.bass_utils import run_bass_kernel_spmd
    x = np.asarray(inputs["x"])
    B, T, D = x.shape
    H = 16
    HD = D // H
    HID = np.asarray(inputs["fc1_w"]).shape[1]
    n_cores = 8
    Bc = B // n_cores
    nc = _get_nc(Bc, T, D, H, HD, HID)
    in_maps = prepare_inputs(inputs, B, T, D, H, HID, n_cores)
    res = run_bass_kernel_spmd(nc, in_maps, list(range(n_cores)), trace=trace)
    out = np.concatenate(
        [res.results[c]["out"].reshape(Bc, T, D) for c in range(n_cores)], axis=0)
    return out, res


def kernel(**inputs) -> np.ndarray:
    out, _ = _run(inputs, trace=False)
    return out.astype(np.float32)
